# revision 48
# baseline (speedup 1.0000x reference)
"""Trainium2 Bass kernel for nn_DEINA: encoder + Koopman linear recurrence.

Self-contained: shards the batch (512 trajectories) over 8 NeuronCores
(64 trajectories each), runs a fused encoder + blocked-scan recurrence
per core, and gathers the full outputs.

Math (per trajectory, T=256 steps, D=64, H=256, G=192, L=256):
    g  = relu(x Wx1 + bx1); g = relu(g Wx2 + bx2); g = g Wx3
    y  = [x, g]                                  (output 1)
    v  = relu(u Wu1 + bu1) Wu2;  uu = [u, v];  Bu = uu WB
    y_pred[0] = y[0];  y_pred[t+1] = y_pred[t] K + bK + Bu[t]   (output 2)

Key optimizations over the straightforward version:
  - Bu is computed without materializing v:  Bu = u WB[:64] + h1u (Wu2 WB[64:])
    with the fused weight WuB = Wu2 @ WB[64:] built once on device.
  - K powers P_j (j=1..16) are chained in f32r (full PE rate) and stored
    as [P_{2k+1} | P_{2k+2}] pairs so phase-3 matmuls run at N=512.
  - The y x-part is written DRAM->DRAM (no SBUF bounce).
  - The time recurrence is a blocked scan with S=16:
      phase 1: z[b,j] = z[b,j-1] K + c[b,j]  batched over blocks
      phase 2: 15 tiny serial steps through K^16
      phase 3: y_pred[b*16+j] = ys[b] K^j + z[b,j-1], two j's per matmul
"""

import numpy as np

import concourse.bacc as bacc
import concourse.bass as bass
import concourse.tile as tile
from concourse import mybir
from concourse.bass import ts
from concourse.bass_utils import run_bass_kernel_spmd
from concourse.masks import make_identity

F32 = mybir.dt.float32
F32R = mybir.dt.float32r
BF16 = mybir.dt.bfloat16
RELU = mybir.ActivationFunctionType.Relu

NCORES = 8
BL = 64  # trajectories per core
T = 256
D = 64
H = 256
G = 192
L = 256
S = 16  # scan block size (= time steps per chunk)
NB = 16  # number of blocks
NG = 4  # block groups
GB = 4  # blocks per group


def _build():
    nc = bacc.Bacc("TRN2", target_bir_lowering=False)

    x_h = nc.dram_tensor("x", [BL, T, D], F32, kind="ExternalInput")
    u_h = nc.dram_tensor("u", [BL, T, D], F32, kind="ExternalInput")
    wx1_h = nc.dram_tensor("Wx1", [D, H], F32, kind="ExternalInput")
    bx1_h = nc.dram_tensor("bx1", [H], F32, kind="ExternalInput")
    wx2_h = nc.dram_tensor("Wx2", [H, H], F32, kind="ExternalInput")
    bx2_h = nc.dram_tensor("bx2", [H], F32, kind="ExternalInput")
    wx3_h = nc.dram_tensor("Wx3", [H, G], F32, kind="ExternalInput")
    wu1_h = nc.dram_tensor("Wu1", [D, H], F32, kind="ExternalInput")
    bu1_h = nc.dram_tensor("bu1", [H], F32, kind="ExternalInput")
    wu2_h = nc.dram_tensor("Wu2", [H, G], F32, kind="ExternalInput")
    wb_h = nc.dram_tensor("WB", [L, L], F32, kind="ExternalInput")
    wk_h = nc.dram_tensor("WK", [L, L], F32, kind="ExternalInput")
    bk_h = nc.dram_tensor("bK", [L], F32, kind="ExternalInput")
    y_h = nc.dram_tensor("y", [BL, T, L], F32, kind="ExternalOutput")
    yp_h = nc.dram_tensor("y_pred", [BL, T, L], F32, kind="ExternalOutput")

    with tile.TileContext(nc) as tc, tile.ExitStack() as ctx:
        wpool = ctx.enter_context(tc.tile_pool(name="w", bufs=1))
        encpool = ctx.enter_context(tc.tile_pool(name="enc", bufs=3))
        inppool = ctx.enter_context(tc.tile_pool(name="inp", bufs=3))
        actpool = ctx.enter_context(tc.tile_pool(name="act", bufs=2))
        czpool = ctx.enter_context(tc.tile_pool(name="cz", bufs=2))
        yspool = ctx.enter_context(tc.tile_pool(name="ys", bufs=2))
        yppool = ctx.enter_context(tc.tile_pool(name="ypd", bufs=5))
        stgpool = ctx.enter_context(tc.tile_pool(name="stg", bufs=1))
        gspool = ctx.enter_context(tc.tile_pool(name="gs", bufs=3))
        encps = ctx.enter_context(tc.tile_pool(name="encps", bufs=2, space="PSUM"))
        tpps = ctx.enter_context(tc.tile_pool(name="tpps", bufs=1, space="PSUM"))
        sps = ctx.enter_context(tc.tile_pool(name="sps", bufs=3, space="PSUM"))

        # ------------------------------------------------------------------
        # Input chunk 0 first so the PE can start transposing ASAP
        # ------------------------------------------------------------------
        sxu_tiles = {}

        def load_sxu(b):
            t = encpool.tile([BL, S, 2 * D], BF16, tag="sxu", name="sxu")
            if b == 0:
                # interleaved half-loads: the first (x, u) half unlocks
                # the first 8 t-slab transposes ~7us earlier than two
                # serialized full-chunk cast-DMAs would
                for q in range(2):
                    nc.gpsimd.dma_start(
                        t[:, ts(q, 8), 0:D], x_h[:, q * 8 : q * 8 + 8, :]
                    )
                    nc.gpsimd.dma_start(
                        t[:, ts(q, 8), D : 2 * D], u_h[:, q * 8 : q * 8 + 8, :]
                    )
            else:
                nc.gpsimd.dma_start(t[:, :, 0:D], x_h[:, ts(b, S), :])
                nc.gpsimd.dma_start(t[:, :, D : 2 * D], u_h[:, ts(b, S), :])
            sxu_tiles[b] = t
            return t

        load_sxu(0)

        # Identities early (chunk-0 transposes need identb64, and the DVE
        # FIFO must not park these casts behind weight casts), plus a run
        # of throwaway PE transposes: the PE clock sits at 1.2 GHz until
        # ~3.4us of sustained activity (HAM); warming it while the input
        # DMAs are in flight makes the real matmuls start at 2.4 GHz.
        ident = wpool.tile([128, 128], F32, tag="ident")
        make_identity(nc, ident[:])
        identrt = wpool.tile([128, 128], F32R, tag="identrt")
        nc.vector.tensor_copy(identrt[:], ident[:])
        identb64 = wpool.tile([64, 64], BF16, tag="identb64")
        nc.vector.tensor_copy(identb64[:], ident[0:64, 0:64])
        identb128 = wpool.tile([128, 128], BF16, tag="identb128")
        nc.vector.tensor_copy(identb128[:], ident[:])

        # ------------------------------------------------------------------
        # Weights / constants (issued from several queues in parallel)
        # ------------------------------------------------------------------
        def load_f32(ap, shape, name, pool=wpool, eng=None):
            t = pool.tile(shape, F32, tag=name, name=name)
            (eng or nc.sync).dma_start(t[:], ap)
            return t

        def to_bf16(src, name):
            t = wpool.tile(list(src.shape), BF16, tag=name, name=name)
            nc.vector.tensor_copy(t[:], src[:])
            return t

        # L1 weights: wx1 lives on partitions 0:64, wu1 on 64:128
        wx1f = load_f32(wx1_h[:, :], [D, H], "wx1f", pool=stgpool)
        wx1b = to_bf16(wx1f, "wx1b")
        wu1f = stgpool.tile([128, H], F32, tag="wu1f")
        nc.scalar.dma_start(wu1f[64:128, :], wu1_h[:, :])
        wu1b = wpool.tile([128, H], BF16, tag="wu1b")
        nc.vector.tensor_copy(wu1b[64:128, :], wu1f[64:128, :])

        wx2b, wx3b = [], []
        for lt in range(2):
            wx2b.append(to_bf16(load_f32(wx2_h.ap()[ts(lt, 128), :], [128, H], f"wx2f{lt}", pool=stgpool), f"wx2b{lt}"))
            wx3b.append(to_bf16(load_f32(wx3_h.ap()[ts(lt, 128), :], [128, G], f"wx3f{lt}", pool=stgpool, eng=nc.scalar), f"wx3b{lt}"))

        # WB split: wb1 = WB[0:64] (u-part, on partitions 64:128);
        # wb2 = WB[64:256] (v-part, fused into WuB below)
        wb1f = stgpool.tile([128, L], F32, tag="wb1f")
        nc.scalar.dma_start(wb1f[64:128, :], wb_h.ap()[0:64, :])
        wb1b = wpool.tile([128, L], BF16, tag="wb1b")
        nc.vector.tensor_copy(wb1b[64:128, :], wb1f[64:128, :])
        wb2f = [
            load_f32(wb_h.ap()[64:192, :], [128, L], "wb2f0", pool=stgpool, eng=nc.scalar),
            load_f32(wb_h.ap()[192:256, :], [64, L], "wb2f1", pool=stgpool, eng=nc.scalar),
        ]
        wu2f = [load_f32(wu2_h.ap()[ts(lt, 128), :], [128, G], f"wu2f{lt}", pool=stgpool, eng=nc.scalar) for lt in range(2)]

        # biases as per-partition scalars: col j holds b[j*128 + p]
        def load_bias(h, name):
            t = wpool.tile([128, 2], F32, tag=name, name=name)
            nc.sync.dma_start(t[:], h.rearrange("(t p) -> p t", p=128))
            return t

        bx1v = load_bias(bx1_h, "bx1v")
        bx2v = load_bias(bx2_h, "bx2v")
        bu1v = load_bias(bu1_h, "bu1v")
        bkv = load_bias(bk_h, "bkv")

        # K tiles (f32)
        kf = [load_f32(wk_h.ap()[ts(lt, 128), :], [128, L], f"kf{lt}") for lt in range(2)]

        # ------------------------------------------------------------------
        # Views for strided HBM I/O
        # ------------------------------------------------------------------
        # g-part of y: one DMA per chunk; rows (j2, traj), free (mt, l)
        yv_g = y_h.rearrange("traj (b mt j2) l -> b j2 traj mt l", b=NB, mt=8, j2=2)
        # y_pred singles (j=0): rows (nb2, traj), free (mt, l)
        ypv = yp_h.rearrange(
            "traj (g mt nb2 j) l -> g j nb2 traj mt l", g=NG, mt=2, nb2=2, j=S
        )
        # y_pred pairs: per (g, mt): dims (nb2, traj, j, l)
        ypv2 = yp_h.rearrange(
            "traj (g mt nb2 j) l -> g mt nb2 traj j l", g=NG, mt=2, nb2=2, j=S
        )

        cz = {}  # (group, lt) -> [128, S, GB, BL] f32 tile
        ys = {}  # (group, lt) -> [128, GB, BL] f32 tile

        def get_cz(g, lt):
            if (g, lt) not in cz:
                cz[(g, lt)] = czpool.tile([128, S, GB, BL], BF16, tag=f"cz{lt}", name=f"cz{g}_{lt}")
            return cz[(g, lt)]

        def get_ys(g, lt):
            if (g, lt) not in ys:
                ys[(g, lt)] = yspool.tile([128, GB, BL], F32R, tag=f"ys{lt}", name=f"ys{g}_{lt}")
            return ys[(g, lt)]

        # ------------------------------------------------------------------
        # Encoder chunk: one block b (16 time steps x 64 trajectories),
        # processed as one 1024-token wave (N=1024 matmuls)
        # ------------------------------------------------------------------
        def drain(gen, n=None):
            if gen is None:
                return
            try:
                if n is None:
                    while True:
                        next(gen)
                else:
                    for _ in range(n):
                        next(gen)
            except StopIteration:
                pass

        def encoder_chunk(b, filler=None, prep_hook=None):
            g = b // GB
            big = b % GB
            sxu = sxu_tiles[b]
            # y x-part: straight DRAM->DRAM passthrough
            nc.gpsimd.dma_start(y_h[:, ts(b, S), 0:D], x_h[:, ts(b, S), :])

            # PE-transpose the 16 [64,128] t-slabs into one psum bank:
            # partitions (x-d | u-d), cols (t, traj)
            xps = tpps.tile([128, S * BL], BF16, tag="tpps", name="tpps_t")
            for t in range(S):
                nc.tensor.matmul(
                    xps[:, ts(t, BL)], sxu[:, t, :], identb64[:],
                    is_transpose=True, start=(t == 0), stop=(t == S - 1),
                )
            xu = encpool.tile([128, S * BL], BF16, tag="xu", name="xu")
            nc.any.tensor_copy(xu[:], xps[:])
            drain(filler, 2)

            rx = xu[0:D, :]
            ru = xu[D:128, :]
            NTOK = S * BL  # 1024

            # L1: h1x = relu(Wx1^T x^T + bx1), h1u likewise (K=64)
            h1xs, h1us = [], []
            for mt in range(2):
                psx = encps.tile([128, NTOK], F32, tag="encps", name="encps_t")
                psu = encps.tile([128, NTOK], F32, tag="encps", name="encps_t")
                for hf in range(2):
                    # x on array rows 0:64, u on rows 64:128 -> concurrent
                    nc.tensor.matmul(
                        psx[:, ts(hf, 512)], wx1b[:, ts(mt, 128)], rx[:, ts(hf, 512)],
                        start=True, stop=True,
                    )
                    nc.tensor.matmul(
                        psu[:, ts(hf, 512)], wu1b[64:128, ts(mt, 128)], ru[:, ts(hf, 512)],
                        start=True, stop=True, tile_position=(64, 0),
                    )
                sbx = actpool.tile([128, NTOK], BF16, tag=f"h1x{mt}", name=f"h1x{mt}_t")
                nc.scalar.activation(sbx[:], psx[:], RELU, bias=bx1v[:, mt : mt + 1])
                h1xs.append(sbx)
                sbu = actpool.tile([128, NTOK], BF16, tag=f"h1u{mt}", name=f"h1u{mt}_t")
                nc.vector.tensor_scalar(
                    sbu[:], psu[:], bu1v[:, mt : mt + 1], 0.0,
                    op0=mybir.AluOpType.add, op1=mybir.AluOpType.max,
                )
                h1us.append(sbu)
                drain(filler, 2)

            # L2: h2x = relu(Wx2^T h1x + bx2)
            h2xs = []
            for mt in range(2):
                ps = encps.tile([128, NTOK], F32, tag="encps", name="encps_t")
                for hf in range(2):
                    for lt in range(2):
                        nc.tensor.matmul(
                            ps[:, ts(hf, 512)], wx2b[lt][:, ts(mt, 128)],
                            h1xs[lt][:, ts(hf, 512)],
                            start=(lt == 0), stop=(lt == 1),
                        )
                sb = actpool.tile([128, NTOK], BF16, tag=f"h2x{mt}", name=f"h2x{mt}_t")
                nc.scalar.activation(sb[:], ps[:], RELU, bias=bx2v[:, mt : mt + 1])
                h2xs.append(sb)
                drain(filler, 2)

            if prep_hook is not None:
                prep_hook()

            # Bu + bK -> c buffer, via fused WuB (no v materialization):
            # Bu = u @ WB[0:64] + h1u @ WuB
            for mt in range(2):
                ps = encps.tile([128, NTOK], F32, tag="encps", name="encps_t")
                for hf in range(2):
                    nc.tensor.matmul(
                        ps[:, ts(hf, 512)], wb1b[64:128, ts(mt, 128)],
                        ru[:, ts(hf, 512)],
                        start=True, stop=False, tile_position=(64, 0),
                    )
                    for lt in range(2):
                        nc.tensor.matmul(
                            ps[:, ts(hf, 512)], wuBb[lt][:, ts(mt, 128)],
                            h1us[lt][:, ts(hf, 512)],
                            start=False, stop=(lt == 1),
                        )
                czt = get_cz(g, mt)
                nc.vector.tensor_scalar_add(
                    czt[:, :, big, :],
                    ps[:].rearrange("p (a c) -> p a c", a=S),
                    bkv[:, mt : mt + 1],
                )
                drain(filler, 2)

            # g (natural layout) = h2x @ Wx3: 8 M-tiles in 2 psum tiles
            gps = [encps.tile([128, NTOK], F32, tag="encps", name="gps_t") for _ in range(2)]
            drain(filler, 1)
            for mt8 in range(8):
                out = gps[mt8 // 4][:, (mt8 % 4) * 256 : (mt8 % 4) * 256 + G]
                for lt in range(2):
                    nc.tensor.matmul(
                        out, h2xs[lt][:, ts(mt8, 128)], wx3b[lt][:],
                        start=(lt == 0), stop=(lt == 1),
                    )
            gs = gspool.tile([128, 8, G], F32, tag="gs", name="gs_t")
            nc.scalar.copy(
                gs[:, ts(0, 4), :],
                gps[0][:].rearrange("p (m x) -> p m x", m=4)[:, :, 0:G],
            )
            nc.vector.tensor_copy(
                gs[:, ts(1, 4), :],
                gps[1][:].rearrange("p (m x) -> p m x", m=4)[:, :, 0:G],
            )
            for j2 in range(2):
                nc.gpsimd.dma_start(yv_g[b, j2][:, :, D:L], gs[ts(j2, 64), :, :])

            # y0 (t = 0): x-part copied from xu, g-part via matmuls
            if b == 0:
                nc.vector.tensor_copy(get_ys(0, 0)[0:D, 0, :], xu[0:D, 0:BL])
                y0a = sps.tile([128, BL], F32, tag="sps", name="y0a_t")
                for lt in range(2):
                    nc.tensor.matmul(
                        y0a[64:128, :], wx3b[lt][:, 0:64], h2xs[lt][:, 0:BL],
                        start=(lt == 0), stop=(lt == 1), tile_position=(0, 64),
                    )
                nc.vector.tensor_copy(get_ys(0, 0)[64:128, 0, :], y0a[64:128, :])
                y0b = sps.tile([128, BL], F32, tag="sps", name="y0b_t")
                for lt in range(2):
                    nc.tensor.matmul(
                        y0b[:], wx3b[lt][:, 64:192], h2xs[lt][:, 0:BL],
                        start=(lt == 0), stop=(lt == 1),
                    )
                nc.vector.tensor_copy(get_ys(0, 1)[:, 0, :], y0b[:])

        # ------------------------------------------------------------------
        # One-time weight prep on PE: K^T, fused WuB, K powers (f32r chain).
        # Emitted before chunk 0 (its Bu needs WuB); the serial power chain
        # is spread between chunk emissions so the in-order PE queue never
        # stalls on its psum->sbuf copy latency.
        # ------------------------------------------------------------------
        # K^T tiles (for the power chain): kT[b][p, a] = K[a, b*128+p]
        kT = [wpool.tile([128, L], F32R, tag=f"kT{lt}", name=f"kT{lt}") for lt in range(2)]
        wu2T0 = stgpool.tile([128, H], F32, tag="wu2T0")
        wu2T1 = stgpool.tile([64, H], F32, tag="wu2T1")
        wuBb = []
        kb1 = [wpool.tile([128, L], BF16, tag=f"kb1_{rt}", name=f"kb1_{rt}") for rt in range(2)]
        prp = {}
        for rt in range(2):
            for k in range(8):
                prp[(k, rt)] = wpool.tile([128, 2 * L], F32R, tag=f"prp{k}_{rt}", name=f"prp{k}_{rt}")

        def one_time_prep():
            for a in range(2):
                for bb in range(2):
                    pst = sps.tile([128, 128], F32, tag="sps", name="pstT_t")
                    nc.tensor.transpose(pst[:], kf[a][:, ts(bb, 128)], ident[:])
                    nc.vector.tensor_copy(kT[bb][:, ts(a, 128)], pst[:])

            # Wu2^T (for the WuB build)
            for ht in range(2):
                p0 = sps.tile([128, 128], F32, tag="sps", name="wu2t_t")
                nc.tensor.transpose(p0[:], wu2f[ht][:, 0:128], ident[:])
                nc.scalar.copy(wu2T0[:, ts(ht, 128)], p0[:])
                p1 = sps.tile([128, 128], F32, tag="sps", name="wu2t_t")
                nc.tensor.transpose(p1[0:64, :], wu2f[ht][:, 128:192], ident[:])
                nc.scalar.copy(wu2T1[:, ts(ht, 128)], p1[0:64, :])

            # WuB = Wu2 @ WB[64:256]  (bf16, 2 row tiles)
            for mt in range(2):
                ps = sps.tile([128, L], F32, tag="sps", name="wuB_t")
                nc.tensor.matmul(
                    ps[:], wu2T0[:, ts(mt, 128)], wb2f[0][:],
                    start=True, stop=False,
                )
                nc.tensor.matmul(
                    ps[:], wu2T1[:, ts(mt, 128)], wb2f[1][:],
                    start=False, stop=True,
                )
                wb_t = wpool.tile([128, L], BF16, tag=f"wuBb{mt}", name=f"wuBb{mt}")
                nc.scalar.copy(wb_t[:], ps[:])
                wuBb.append(wb_t)

            # K powers pair-store init: P_1 = K; kb1 = bf16 K for phase 1
            for rt in range(2):
                nc.vector.tensor_copy(prp[(0, rt)][:, 0:L], kf[rt][:])
                nc.scalar.copy(kb1[rt][:], kf[rt][:])

        def pslice(j, rt):  # P_j for row-tile rt
            k, c = (j - 1) // 2, (j - 1) % 2
            return prp[(k, rt)][:, c * L : (c + 1) * L]

        def emit_chain(j0, j1):
            for j in range(j0, j1):
                for rt in range(2):
                    pst = sps.tile([128, L], F32, tag="sps", name="pstP_t")
                    for bt in range(2):
                        nc.tensor.matmul(
                            pst[:],
                            kT[bt][:, ts(rt, 128)],
                            pslice(j - 1, bt),
                            start=(bt == 0),
                            stop=(bt == 1),
                        )
                    nc.vector.tensor_copy(pslice(j, rt), pst[:])

        # ------------------------------------------------------------------
        # Phase 1: batched local scans (per group)
        # ------------------------------------------------------------------
        def phase1_gen(g):
            czt = [get_cz(g, lt) for lt in range(2)]
            for j in range(1, S):
                zprev = [czt[lt][:, j - 1, :, :].rearrange("p a c -> p (a c)") for lt in range(2)]
                ps = sps.tile([128, 512], F32, tag="sps", name="p1ps_t")
                for l2t in range(2):
                    for l1t in range(2):
                        nc.tensor.matmul(
                            ps[:, ts(l2t, GB * BL)],
                            kb1[l1t][:, ts(l2t, 128)],
                            zprev[l1t],
                            start=(l1t == 0 and l2t == 0),
                            stop=(l1t == 1 and l2t == 1),
                        )
                for l2t in range(2):
                    nc.vector.tensor_add(
                        czt[l2t][:, j, :, :],
                        ps[:, ts(l2t, GB * BL)].rearrange("p (b c) -> p b c", b=GB),
                        czt[l2t][:, j, :, :],
                    )
                yield

        # ------------------------------------------------------------------
        # Phase 2: block-level scan (serial, 4 steps per group)
        # ------------------------------------------------------------------
        def p2step(g, nb):
            b = g * GB + nb
            if b >= NB - 1:
                return
            ng, nnb = (g, nb + 1) if nb + 1 < GB else (g + 1, 0)
            ps = sps.tile([128, 2 * BL], F32, tag="sps", name="p2ps_t")
            for lt in range(2):
                for l1t in range(2):
                    nc.tensor.matmul(
                        ps[:, ts(lt, BL)],
                        pslice(S, l1t)[:, ts(lt, 128)],
                        get_ys(g, l1t)[:, nb, :],
                        start=(l1t == 0 and lt == 0),
                        stop=(l1t == 1 and lt == 1),
                    )
            for lt in range(2):
                nc.vector.tensor_add(
                    get_ys(ng, lt)[:, nnb, :], ps[:, ts(lt, BL)],
                    get_cz(g, lt)[:, S - 1, nb, :],
                )

        # ------------------------------------------------------------------
        # Phase 3: fix-up, natural-layout output.
        # j = 0: transpose-only.  j in {1..14}: pairs (2k+1, 2k+2) with
        # N=512 matmuls against the prp pair tiles.  j = 15: single.
        # ------------------------------------------------------------------
        def p3_j0(g, mt):
            ps0 = sps.tile([128, L], F32, tag="sps", name="p3ps0_t")
            for lt in range(2):
                nc.tensor.matmul(
                    ps0[:, lt * 128 : lt * 128 + 128].bitcast(F32R),
                    get_ys(g, lt)[:, ts(mt, 2), :].rearrange("p a c -> p (a c)"),
                    identrt[:],
                    is_transpose=True, start=(lt == 0), stop=(lt == 1),
                )
            ysb0 = yppool.tile([128, L], F32, tag="ysbs", name="ysb0_t")
            nc.vector.tensor_copy(ysb0[:], ps0[:])
            for nb2 in range(2):
                eng = nc.sync if nb2 == 0 else nc.scalar
                eng.dma_start(ypv[g, 0][nb2][:, mt, :], ysb0[ts(nb2, 64), :])

        def p3_pair(g, k, mt):
            ps = sps.tile([128, 2 * L], F32, tag="sps", name="p3ps_t")
            for l1t in range(2):
                nc.tensor.matmul(
                    ps[:],
                    get_ys(g, l1t)[:, ts(mt, 2), :].rearrange("p a c -> p (a c)"),
                    prp[(k, l1t)][:],
                    start=(l1t == 0), stop=False,
                )
            for jc in range(2):
                for lt in range(2):
                    nc.tensor.matmul(
                        ps[:, jc * L + lt * 128 : jc * L + lt * 128 + 128],
                        get_cz(g, lt)[:, 2 * k + jc, ts(mt, 2), :].rearrange("p a c -> p (a c)"),
                        identb128[:],
                        start=False, stop=(jc == 1 and lt == 1),
                    )
            ysb = yppool.tile([128, 2, L], F32, tag="ysb", name="ysb_t")
            nc.scalar.copy(ysb[:], ps[:].rearrange("p (m x) -> p m x", m=2))
            for nb2 in range(2):
                eng = nc.sync if nb2 == 0 else nc.scalar
                eng.dma_start(
                    ypv2[g, mt][nb2][:, 2 * k + 1 : 2 * k + 3, :],
                    ysb[ts(nb2, 64), :, :],
                )

        def p3_j15(g, mt):
            psf = sps.tile([128, L], F32, tag="sps", name="p3psf_t")
            for l1t in range(2):
                nc.tensor.matmul(
                    psf[:],
                    get_ys(g, l1t)[:, ts(mt, 2), :].rearrange("p a c -> p (a c)"),
                    pslice(S - 1, l1t),
                    start=(l1t == 0), stop=False,
                )
            for lt in range(2):
                nc.tensor.matmul(
                    psf[:, lt * 128 : lt * 128 + 128],
                    get_cz(g, lt)[:, S - 2, ts(mt, 2), :].rearrange("p a c -> p (a c)"),
                    identb128[:],
                    start=False, stop=(lt == 1),
                )
            ysbf = yppool.tile([128, L], F32, tag="ysbs", name="ysbf_t")
            nc.vector.tensor_copy(ysbf[:], psf[:])
            for nb2 in range(2):
                eng = nc.sync if nb2 == 0 else nc.scalar
                eng.dma_start(ypv[g, S - 1][nb2][:, mt, :], ysbf[ts(nb2, 64), :])

        def phase23(g, filler=None):
            # block-pair mt=0 needs ys blocks 0,1 (ready after p2 step 0);
            # mt=1 needs blocks 2,3 (after steps 1,2).  p2 step 3 rolls the
            # carry into the next group.  The filler (an interleaved phase-1
            # chain) fills the PE stalls between serial p2 steps.
            p2step(g, 0)
            drain(filler, 1)
            p3_j0(g, 0)
            drain(filler, 1)
            for k in range(7):
                p3_pair(g, k, 0)
                drain(filler, 1)
            p3_j15(g, 0)
            drain(filler, 1)
            p2step(g, 1)
            drain(filler, 1)
            p2step(g, 2)
            drain(filler, 1)
            p3_j0(g, 1)
            drain(filler, 1)
            for k in range(7):
                p3_pair(g, k, 1)
                drain(filler, 1)
            p3_j15(g, 1)
            p2step(g, 3)
            drain(filler)

        # ------------------------------------------------------------------
        # Emit: chunk 0 first (PE starts ASAP), power-chain steps spread
        # between chunks, recurrence phases pipelined one group behind.
        # ------------------------------------------------------------------
        encoder_chunk(0, prep_hook=one_time_prep)
        for b in range(1, GB):
            load_sxu(b)
        emit_chain(2, 7)
        encoder_chunk(1)
        emit_chain(7, 12)
        encoder_chunk(2)
        emit_chain(12, S + 1)
        encoder_chunk(3)
        for g in range(1, NG):
            for big in range(GB):
                load_sxu(g * GB + big)
            for big in range(GB - 1):
                encoder_chunk(g * GB + big)
            f = phase1_gen(g - 1)
            encoder_chunk(g * GB + GB - 1, filler=f)
            drain(f)
            if g < NG - 1:
                phase23(g - 1)
            else:
                phase23(g - 1, filler=phase1_gen(NG - 1))
        phase23(NG - 1)

    nc.compile()
    return nc


_NC = None


def _get_nc():
    global _NC
    if _NC is None:
        _NC = _build()
    return _NC


def kernel(**inputs):
    nc = _get_nc()
    wnames = [
        "Wx1", "bx1", "Wx2", "bx2", "Wx3", "Wu1", "bu1", "Wu2", "WB", "WK", "bK",
    ]
    weights = {k: np.ascontiguousarray(np.asarray(inputs[k], dtype=np.float32)) for k in wnames}
    x = np.asarray(inputs["x"], dtype=np.float32)
    u = np.asarray(inputs["u"], dtype=np.float32)
    in_maps = []
    for c in range(NCORES):
        m = dict(weights)
        m["x"] = np.ascontiguousarray(x[c * BL : (c + 1) * BL])
        m["u"] = np.ascontiguousarray(u[c * BL : (c + 1) * BL])
        in_maps.append(m)
    res = run_bass_kernel_spmd(nc, in_maps, core_ids=list(range(NCORES)))
    y = np.concatenate([r["y"] for r in res.results], axis=0)
    y_pred = np.concatenate([r["y_pred"] for r in res.results], axis=0)
    return (y, y_pred)



# revision 49
# speedup vs baseline: 1.1874x; 1.1874x over previous
"""Trainium2 Bass kernel for nn_DEINA: encoder + Koopman linear recurrence.

Self-contained: shards the batch (512 trajectories) over 8 NeuronCores
(64 trajectories each), runs a fused encoder + blocked-scan recurrence
per core, and gathers the full outputs.

Math (per trajectory, T=256 steps, D=64, H=256, G=192, L=256):
    g  = relu(x Wx1 + bx1); g = relu(g Wx2 + bx2); g = g Wx3
    y  = [x, g]                                  (output 1)
    v  = relu(u Wu1 + bu1) Wu2;  uu = [u, v];  Bu = uu WB
    y_pred[0] = y[0];  y_pred[t+1] = y_pred[t] K + bK + Bu[t]   (output 2)

Key optimizations over the straightforward version:
  - Bu is computed without materializing v:  Bu = u WB[:64] + h1u (Wu2 WB[64:])
    with the fused weight WuB = Wu2 @ WB[64:] built once on device.
  - K powers P_j (j=1..16) are chained in f32r (full PE rate) and stored
    as [P_{2k+1} | P_{2k+2}] pairs so phase-3 matmuls run at N=512.
  - The y x-part is written DRAM->DRAM (no SBUF bounce).
  - The time recurrence is a blocked scan with S=16:
      phase 1: z[b,j] = z[b,j-1] K + c[b,j]  batched over blocks
      phase 2: 15 tiny serial steps through K^16
      phase 3: y_pred[b*16+j] = ys[b] K^j + z[b,j-1], two j's per matmul
"""

import numpy as np

import concourse.bacc as bacc
import concourse.bass as bass
import concourse.tile as tile
from concourse import mybir
from concourse.bass import ts
from concourse.bass_utils import run_bass_kernel_spmd
from concourse.masks import make_identity

F32 = mybir.dt.float32
F32R = mybir.dt.float32r
BF16 = mybir.dt.bfloat16
RELU = mybir.ActivationFunctionType.Relu

NCORES = 8
BL = 64  # trajectories per core
T = 256
D = 64
H = 256
G = 192
L = 256
S = 16  # scan block size (= time steps per chunk)
NB = 16  # number of blocks
NG = 4  # block groups
GB = 4  # blocks per group


def _build():
    nc = bacc.Bacc("TRN2", target_bir_lowering=False)

    x_h = nc.dram_tensor("x", [BL, T, D], F32, kind="ExternalInput")
    u_h = nc.dram_tensor("u", [BL, T, D], F32, kind="ExternalInput")
    wx1_h = nc.dram_tensor("Wx1", [D, H], F32, kind="ExternalInput")
    bx1_h = nc.dram_tensor("bx1", [H], F32, kind="ExternalInput")
    wx2_h = nc.dram_tensor("Wx2", [H, H], F32, kind="ExternalInput")
    bx2_h = nc.dram_tensor("bx2", [H], F32, kind="ExternalInput")
    wx3_h = nc.dram_tensor("Wx3", [H, G], F32, kind="ExternalInput")
    wu1_h = nc.dram_tensor("Wu1", [D, H], F32, kind="ExternalInput")
    bu1_h = nc.dram_tensor("bu1", [H], F32, kind="ExternalInput")
    wu2_h = nc.dram_tensor("Wu2", [H, G], F32, kind="ExternalInput")
    wb_h = nc.dram_tensor("WB", [L, L], F32, kind="ExternalInput")
    wk_h = nc.dram_tensor("WK", [L, L], F32, kind="ExternalInput")
    bk_h = nc.dram_tensor("bK", [L], F32, kind="ExternalInput")
    y_h = nc.dram_tensor("y", [BL, T, L], F32, kind="ExternalOutput")
    yp_h = nc.dram_tensor("y_pred", [BL, T, L], F32, kind="ExternalOutput")

    with tile.TileContext(nc) as tc, tile.ExitStack() as ctx:
        wpool = ctx.enter_context(tc.tile_pool(name="w", bufs=1))
        encpool = ctx.enter_context(tc.tile_pool(name="enc", bufs=3))
        inppool = ctx.enter_context(tc.tile_pool(name="inp", bufs=3))
        actpool = ctx.enter_context(tc.tile_pool(name="act", bufs=2))
        czpool = ctx.enter_context(tc.tile_pool(name="cz", bufs=2))
        yspool = ctx.enter_context(tc.tile_pool(name="ys", bufs=2))
        yppool = ctx.enter_context(tc.tile_pool(name="ypd", bufs=5))
        stgpool = ctx.enter_context(tc.tile_pool(name="stg", bufs=1))
        gspool = ctx.enter_context(tc.tile_pool(name="gs", bufs=3))
        encps = ctx.enter_context(tc.tile_pool(name="encps", bufs=2, space="PSUM"))
        tpps = ctx.enter_context(tc.tile_pool(name="tpps", bufs=1, space="PSUM"))
        sps = ctx.enter_context(tc.tile_pool(name="sps", bufs=3, space="PSUM"))

        # ------------------------------------------------------------------
        # Input chunk 0 first so the PE can start transposing ASAP
        # ------------------------------------------------------------------
        sxu_tiles = {}

        def load_sxu(b):
            t = encpool.tile([BL, S, 2 * D], BF16, tag="sxu", name="sxu")
            nc.gpsimd.dma_start(t[:, :, 0:D], x_h[:, ts(b, S), :])
            nc.gpsimd.dma_start(t[:, :, D : 2 * D], u_h[:, ts(b, S), :])
            sxu_tiles[b] = t
            return t

        load_sxu(0)

        # Identities early (chunk-0 transposes need identb64, and the DVE
        # FIFO must not park these casts behind weight casts), plus a run
        # of throwaway PE transposes: the PE clock sits at 1.2 GHz until
        # ~3.4us of sustained activity (HAM); warming it while the input
        # DMAs are in flight makes the real matmuls start at 2.4 GHz.
        ident = wpool.tile([128, 128], F32, tag="ident")
        make_identity(nc, ident[:])
        identrt = wpool.tile([128, 128], F32R, tag="identrt")
        nc.vector.tensor_copy(identrt[:], ident[:])
        identb64 = wpool.tile([64, 64], BF16, tag="identb64")
        nc.vector.tensor_copy(identb64[:], ident[0:64, 0:64])
        identb128 = wpool.tile([128, 128], BF16, tag="identb128")
        nc.vector.tensor_copy(identb128[:], ident[:])

        # ------------------------------------------------------------------
        # Weights / constants (issued from several queues in parallel)
        # ------------------------------------------------------------------
        def load_f32(ap, shape, name, pool=wpool, eng=None):
            t = pool.tile(shape, F32, tag=name, name=name)
            (eng or nc.sync).dma_start(t[:], ap)
            return t

        def to_bf16(src, name):
            t = wpool.tile(list(src.shape), BF16, tag=name, name=name)
            nc.vector.tensor_copy(t[:], src[:])
            return t

        # L1 weights: wx1 lives on partitions 0:64, wu1 on 64:128
        wx1f = load_f32(wx1_h[:, :], [D, H], "wx1f", pool=stgpool)
        wx1b = to_bf16(wx1f, "wx1b")
        wu1f = stgpool.tile([128, H], F32, tag="wu1f")
        nc.scalar.dma_start(wu1f[64:128, :], wu1_h[:, :])
        wu1b = wpool.tile([128, H], BF16, tag="wu1b")
        nc.vector.tensor_copy(wu1b[64:128, :], wu1f[64:128, :])

        wx2b, wx3b = [], []
        for lt in range(2):
            wx2b.append(to_bf16(load_f32(wx2_h.ap()[ts(lt, 128), :], [128, H], f"wx2f{lt}", pool=stgpool), f"wx2b{lt}"))
            wx3b.append(to_bf16(load_f32(wx3_h.ap()[ts(lt, 128), :], [128, G], f"wx3f{lt}", pool=stgpool, eng=nc.scalar), f"wx3b{lt}"))

        # WB split: wb1 = WB[0:64] (u-part, on partitions 64:128);
        # wb2 = WB[64:256] (v-part, fused into WuB below)
        wb1f = stgpool.tile([128, L], F32, tag="wb1f")
        nc.scalar.dma_start(wb1f[64:128, :], wb_h.ap()[0:64, :])
        wb1b = wpool.tile([128, L], BF16, tag="wb1b")
        nc.vector.tensor_copy(wb1b[64:128, :], wb1f[64:128, :])
        wb2f = [
            load_f32(wb_h.ap()[64:192, :], [128, L], "wb2f0", pool=stgpool, eng=nc.scalar),
            load_f32(wb_h.ap()[192:256, :], [64, L], "wb2f1", pool=stgpool, eng=nc.scalar),
        ]
        wu2f = [load_f32(wu2_h.ap()[ts(lt, 128), :], [128, G], f"wu2f{lt}", pool=stgpool, eng=nc.scalar) for lt in range(2)]

        # biases as per-partition scalars: col j holds b[j*128 + p]
        def load_bias(h, name):
            t = wpool.tile([128, 2], F32, tag=name, name=name)
            nc.sync.dma_start(t[:], h.rearrange("(t p) -> p t", p=128))
            return t

        bx1v = load_bias(bx1_h, "bx1v")
        bx2v = load_bias(bx2_h, "bx2v")
        bu1v = load_bias(bu1_h, "bu1v")
        bkv = load_bias(bk_h, "bkv")

        # K tiles (f32)
        kf = [load_f32(wk_h.ap()[ts(lt, 128), :], [128, L], f"kf{lt}") for lt in range(2)]

        # ------------------------------------------------------------------
        # Views for strided HBM I/O
        # ------------------------------------------------------------------
        # g-part of y: one DMA per chunk; rows (j2, traj), free (mt, l)
        yv_g = y_h.rearrange("traj (b mt j2) l -> b j2 traj mt l", b=NB, mt=8, j2=2)
        # y_pred singles (j=0): rows (nb2, traj), free (mt, l)
        ypv = yp_h.rearrange(
            "traj (g mt nb2 j) l -> g j nb2 traj mt l", g=NG, mt=2, nb2=2, j=S
        )
        # y_pred pairs: per (g, mt): dims (nb2, traj, j, l)
        ypv2 = yp_h.rearrange(
            "traj (g mt nb2 j) l -> g mt nb2 traj j l", g=NG, mt=2, nb2=2, j=S
        )

        cz = {}  # (group, lt) -> [128, S, GB, BL] f32 tile
        ys = {}  # (group, lt) -> [128, GB, BL] f32 tile

        def get_cz(g, lt):
            if (g, lt) not in cz:
                cz[(g, lt)] = czpool.tile([128, S, GB, BL], BF16, tag=f"cz{lt}", name=f"cz{g}_{lt}")
            return cz[(g, lt)]

        def get_ys(g, lt):
            if (g, lt) not in ys:
                ys[(g, lt)] = yspool.tile([128, GB, BL], F32R, tag=f"ys{lt}", name=f"ys{g}_{lt}")
            return ys[(g, lt)]

        # ------------------------------------------------------------------
        # Encoder chunk: one block b (16 time steps x 64 trajectories),
        # processed as one 1024-token wave (N=1024 matmuls)
        # ------------------------------------------------------------------
        def drain(gen, n=None):
            if gen is None:
                return
            try:
                if n is None:
                    while True:
                        next(gen)
                else:
                    for _ in range(n):
                        next(gen)
            except StopIteration:
                pass

        def encoder_chunk(b, filler=None, prep_hook=None):
            g = b // GB
            big = b % GB
            sxu = sxu_tiles[b]
            # y x-part: straight DRAM->DRAM passthrough
            nc.gpsimd.dma_start(y_h[:, ts(b, S), 0:D], x_h[:, ts(b, S), :])

            # PE-transpose the 16 [64,128] t-slabs into one psum bank:
            # partitions (x-d | u-d), cols (t, traj)
            xps = tpps.tile([128, S * BL], BF16, tag="tpps", name="tpps_t")
            for t in range(S):
                nc.tensor.matmul(
                    xps[:, ts(t, BL)], sxu[:, t, :], identb64[:],
                    is_transpose=True, start=(t == 0), stop=(t == S - 1),
                )
            xu = encpool.tile([128, S * BL], BF16, tag="xu", name="xu")
            nc.any.tensor_copy(xu[:], xps[:])
            drain(filler, 2)

            rx = xu[0:D, :]
            ru = xu[D:128, :]
            NTOK = S * BL  # 1024

            # L1: h1x = relu(Wx1^T x^T + bx1), h1u likewise (K=64)
            h1xs, h1us = [], []
            for mt in range(2):
                psx = encps.tile([128, NTOK], F32, tag="encps", name="encps_t")
                psu = encps.tile([128, NTOK], F32, tag="encps", name="encps_t")
                for hf in range(2):
                    # x on array rows 0:64, u on rows 64:128 -> concurrent
                    nc.tensor.matmul(
                        psx[:, ts(hf, 512)], wx1b[:, ts(mt, 128)], rx[:, ts(hf, 512)],
                        start=True, stop=True,
                    )
                    nc.tensor.matmul(
                        psu[:, ts(hf, 512)], wu1b[64:128, ts(mt, 128)], ru[:, ts(hf, 512)],
                        start=True, stop=True, tile_position=(64, 0),
                    )
                sbx = actpool.tile([128, NTOK], BF16, tag=f"h1x{mt}", name=f"h1x{mt}_t")
                nc.scalar.activation(sbx[:], psx[:], RELU, bias=bx1v[:, mt : mt + 1])
                h1xs.append(sbx)
                sbu = actpool.tile([128, NTOK], BF16, tag=f"h1u{mt}", name=f"h1u{mt}_t")
                nc.vector.tensor_scalar(
                    sbu[:], psu[:], bu1v[:, mt : mt + 1], 0.0,
                    op0=mybir.AluOpType.add, op1=mybir.AluOpType.max,
                )
                h1us.append(sbu)
                drain(filler, 2)

            # L2: h2x = relu(Wx2^T h1x + bx2)
            h2xs = []
            for mt in range(2):
                ps = encps.tile([128, NTOK], F32, tag="encps", name="encps_t")
                for hf in range(2):
                    for lt in range(2):
                        nc.tensor.matmul(
                            ps[:, ts(hf, 512)], wx2b[lt][:, ts(mt, 128)],
                            h1xs[lt][:, ts(hf, 512)],
                            start=(lt == 0), stop=(lt == 1),
                        )
                sb = actpool.tile([128, NTOK], BF16, tag=f"h2x{mt}", name=f"h2x{mt}_t")
                nc.scalar.activation(sb[:], ps[:], RELU, bias=bx2v[:, mt : mt + 1])
                h2xs.append(sb)
                drain(filler, 2)

            if prep_hook is not None:
                prep_hook()

            # Bu + bK -> c buffer, via fused WuB (no v materialization):
            # Bu = u @ WB[0:64] + h1u @ WuB
            for mt in range(2):
                ps = encps.tile([128, NTOK], F32, tag="encps", name="encps_t")
                for hf in range(2):
                    nc.tensor.matmul(
                        ps[:, ts(hf, 512)], wb1b[64:128, ts(mt, 128)],
                        ru[:, ts(hf, 512)],
                        start=True, stop=False, tile_position=(64, 0),
                    )
                    for lt in range(2):
                        nc.tensor.matmul(
                            ps[:, ts(hf, 512)], wuBb[lt][:, ts(mt, 128)],
                            h1us[lt][:, ts(hf, 512)],
                            start=False, stop=(lt == 1),
                        )
                czt = get_cz(g, mt)
                nc.vector.tensor_scalar_add(
                    czt[:, :, big, :],
                    ps[:].rearrange("p (a c) -> p a c", a=S),
                    bkv[:, mt : mt + 1],
                )
                drain(filler, 2)

            # g (natural layout) = h2x @ Wx3: 8 M-tiles in 2 psum tiles
            gps = [encps.tile([128, NTOK], F32, tag="encps", name="gps_t") for _ in range(2)]
            drain(filler, 1)
            for mt8 in range(8):
                out = gps[mt8 // 4][:, (mt8 % 4) * 256 : (mt8 % 4) * 256 + G]
                for lt in range(2):
                    nc.tensor.matmul(
                        out, h2xs[lt][:, ts(mt8, 128)], wx3b[lt][:],
                        start=(lt == 0), stop=(lt == 1),
                    )
            gs = gspool.tile([128, 8, G], F32, tag="gs", name="gs_t")
            nc.scalar.copy(
                gs[:, ts(0, 4), :],
                gps[0][:].rearrange("p (m x) -> p m x", m=4)[:, :, 0:G],
            )
            nc.vector.tensor_copy(
                gs[:, ts(1, 4), :],
                gps[1][:].rearrange("p (m x) -> p m x", m=4)[:, :, 0:G],
            )
            for j2 in range(2):
                nc.gpsimd.dma_start(yv_g[b, j2][:, :, D:L], gs[ts(j2, 64), :, :])

            # y0 (t = 0): x-part copied from xu, g-part via matmuls
            if b == 0:
                nc.vector.tensor_copy(get_ys(0, 0)[0:D, 0, :], xu[0:D, 0:BL])
                y0a = sps.tile([128, BL], F32, tag="sps", name="y0a_t")
                for lt in range(2):
                    nc.tensor.matmul(
                        y0a[64:128, :], wx3b[lt][:, 0:64], h2xs[lt][:, 0:BL],
                        start=(lt == 0), stop=(lt == 1), tile_position=(0, 64),
                    )
                nc.vector.tensor_copy(get_ys(0, 0)[64:128, 0, :], y0a[64:128, :])
                y0b = sps.tile([128, BL], F32, tag="sps", name="y0b_t")
                for lt in range(2):
                    nc.tensor.matmul(
                        y0b[:], wx3b[lt][:, 64:192], h2xs[lt][:, 0:BL],
                        start=(lt == 0), stop=(lt == 1),
                    )
                nc.vector.tensor_copy(get_ys(0, 1)[:, 0, :], y0b[:])

        # ------------------------------------------------------------------
        # One-time weight prep on PE: K^T, fused WuB, K powers (f32r chain).
        # Emitted before chunk 0 (its Bu needs WuB); the serial power chain
        # is spread between chunk emissions so the in-order PE queue never
        # stalls on its psum->sbuf copy latency.
        # ------------------------------------------------------------------
        # K^T tiles (for the power chain): kT[b][p, a] = K[a, b*128+p]
        kT = [wpool.tile([128, L], F32R, tag=f"kT{lt}", name=f"kT{lt}") for lt in range(2)]
        wu2T0 = stgpool.tile([128, H], F32, tag="wu2T0")
        wu2T1 = stgpool.tile([64, H], F32, tag="wu2T1")
        wuBb = []
        kb1 = [wpool.tile([128, L], BF16, tag=f"kb1_{rt}", name=f"kb1_{rt}") for rt in range(2)]
        prp = {}
        for rt in range(2):
            for k in range(8):
                prp[(k, rt)] = wpool.tile([128, 2 * L], F32R, tag=f"prp{k}_{rt}", name=f"prp{k}_{rt}")

        def one_time_prep():
            for a in range(2):
                for bb in range(2):
                    pst = sps.tile([128, 128], F32, tag="sps", name="pstT_t")
                    nc.tensor.transpose(pst[:], kf[a][:, ts(bb, 128)], ident[:])
                    nc.vector.tensor_copy(kT[bb][:, ts(a, 128)], pst[:])

            # Wu2^T (for the WuB build)
            for ht in range(2):
                p0 = sps.tile([128, 128], F32, tag="sps", name="wu2t_t")
                nc.tensor.transpose(p0[:], wu2f[ht][:, 0:128], ident[:])
                nc.scalar.copy(wu2T0[:, ts(ht, 128)], p0[:])
                p1 = sps.tile([128, 128], F32, tag="sps", name="wu2t_t")
                nc.tensor.transpose(p1[0:64, :], wu2f[ht][:, 128:192], ident[:])
                nc.scalar.copy(wu2T1[:, ts(ht, 128)], p1[0:64, :])

            # WuB = Wu2 @ WB[64:256]  (bf16, 2 row tiles)
            for mt in range(2):
                ps = sps.tile([128, L], F32, tag="sps", name="wuB_t")
                nc.tensor.matmul(
                    ps[:], wu2T0[:, ts(mt, 128)], wb2f[0][:],
                    start=True, stop=False,
                )
                nc.tensor.matmul(
                    ps[:], wu2T1[:, ts(mt, 128)], wb2f[1][:],
                    start=False, stop=True,
                )
                wb_t = wpool.tile([128, L], BF16, tag=f"wuBb{mt}", name=f"wuBb{mt}")
                nc.scalar.copy(wb_t[:], ps[:])
                wuBb.append(wb_t)

            # K powers pair-store init: P_1 = K; kb1 = bf16 K for phase 1
            for rt in range(2):
                nc.vector.tensor_copy(prp[(0, rt)][:, 0:L], kf[rt][:])
                nc.scalar.copy(kb1[rt][:], kf[rt][:])

        def pslice(j, rt):  # P_j for row-tile rt
            k, c = (j - 1) // 2, (j - 1) % 2
            return prp[(k, rt)][:, c * L : (c + 1) * L]

        def emit_chain(j0, j1):
            for j in range(j0, j1):
                for rt in range(2):
                    pst = sps.tile([128, L], F32, tag="sps", name="pstP_t")
                    for bt in range(2):
                        nc.tensor.matmul(
                            pst[:],
                            kT[bt][:, ts(rt, 128)],
                            pslice(j - 1, bt),
                            start=(bt == 0),
                            stop=(bt == 1),
                        )
                    nc.vector.tensor_copy(pslice(j, rt), pst[:])

        # ------------------------------------------------------------------
        # Phase 1: batched local scans (per group)
        # ------------------------------------------------------------------
        def phase1_gen(g):
            czt = [get_cz(g, lt) for lt in range(2)]
            for j in range(1, S):
                zprev = [czt[lt][:, j - 1, :, :].rearrange("p a c -> p (a c)") for lt in range(2)]
                ps = sps.tile([128, 512], F32, tag="sps", name="p1ps_t")
                for l2t in range(2):
                    for l1t in range(2):
                        nc.tensor.matmul(
                            ps[:, ts(l2t, GB * BL)],
                            kb1[l1t][:, ts(l2t, 128)],
                            zprev[l1t],
                            start=(l1t == 0 and l2t == 0),
                            stop=(l1t == 1 and l2t == 1),
                        )
                for l2t in range(2):
                    nc.vector.tensor_add(
                        czt[l2t][:, j, :, :],
                        ps[:, ts(l2t, GB * BL)].rearrange("p (b c) -> p b c", b=GB),
                        czt[l2t][:, j, :, :],
                    )
                yield

        # ------------------------------------------------------------------
        # Phase 2: block-level scan (serial, 4 steps per group)
        # ------------------------------------------------------------------
        def p2step(g, nb):
            b = g * GB + nb
            if b >= NB - 1:
                return
            ng, nnb = (g, nb + 1) if nb + 1 < GB else (g + 1, 0)
            ps = sps.tile([128, 2 * BL], F32, tag="sps", name="p2ps_t")
            for lt in range(2):
                for l1t in range(2):
                    nc.tensor.matmul(
                        ps[:, ts(lt, BL)],
                        pslice(S, l1t)[:, ts(lt, 128)],
                        get_ys(g, l1t)[:, nb, :],
                        start=(l1t == 0 and lt == 0),
                        stop=(l1t == 1 and lt == 1),
                    )
            for lt in range(2):
                nc.vector.tensor_add(
                    get_ys(ng, lt)[:, nnb, :], ps[:, ts(lt, BL)],
                    get_cz(g, lt)[:, S - 1, nb, :],
                )

        # ------------------------------------------------------------------
        # Phase 3: fix-up, natural-layout output.
        # j = 0: transpose-only.  j in {1..14}: pairs (2k+1, 2k+2) with
        # N=512 matmuls against the prp pair tiles.  j = 15: single.
        # ------------------------------------------------------------------
        def p3_j0(g, mt):
            ps0 = sps.tile([128, L], F32, tag="sps", name="p3ps0_t")
            for lt in range(2):
                nc.tensor.matmul(
                    ps0[:, lt * 128 : lt * 128 + 128].bitcast(F32R),
                    get_ys(g, lt)[:, ts(mt, 2), :].rearrange("p a c -> p (a c)"),
                    identrt[:],
                    is_transpose=True, start=(lt == 0), stop=(lt == 1),
                )
            ysb0 = yppool.tile([128, L], F32, tag="ysbs", name="ysb0_t")
            nc.vector.tensor_copy(ysb0[:], ps0[:])
            for nb2 in range(2):
                eng = nc.scalar if (nb2 == 1 and g == NG - 1) else nc.sync
                eng.dma_start(ypv[g, 0][nb2][:, mt, :], ysb0[ts(nb2, 64), :])

        def p3_pair(g, k, mt):
            ps = sps.tile([128, 2 * L], F32, tag="sps", name="p3ps_t")
            for l1t in range(2):
                nc.tensor.matmul(
                    ps[:],
                    get_ys(g, l1t)[:, ts(mt, 2), :].rearrange("p a c -> p (a c)"),
                    prp[(k, l1t)][:],
                    start=(l1t == 0), stop=False,
                )
            for jc in range(2):
                for lt in range(2):
                    nc.tensor.matmul(
                        ps[:, jc * L + lt * 128 : jc * L + lt * 128 + 128],
                        get_cz(g, lt)[:, 2 * k + jc, ts(mt, 2), :].rearrange("p a c -> p (a c)"),
                        identb128[:],
                        start=False, stop=(jc == 1 and lt == 1),
                    )
            ysb = yppool.tile([128, 2, L], F32, tag="ysb", name="ysb_t")
            nc.scalar.copy(ysb[:], ps[:].rearrange("p (m x) -> p m x", m=2))
            for nb2 in range(2):
                eng = nc.scalar if (nb2 == 1 and g == NG - 1) else nc.sync
                eng.dma_start(
                    ypv2[g, mt][nb2][:, 2 * k + 1 : 2 * k + 3, :],
                    ysb[ts(nb2, 64), :, :],
                )

        def p3_j15(g, mt):
            psf = sps.tile([128, L], F32, tag="sps", name="p3psf_t")
            for l1t in range(2):
                nc.tensor.matmul(
                    psf[:],
                    get_ys(g, l1t)[:, ts(mt, 2), :].rearrange("p a c -> p (a c)"),
                    pslice(S - 1, l1t),
                    start=(l1t == 0), stop=False,
                )
            for lt in range(2):
                nc.tensor.matmul(
                    psf[:, lt * 128 : lt * 128 + 128],
                    get_cz(g, lt)[:, S - 2, ts(mt, 2), :].rearrange("p a c -> p (a c)"),
                    identb128[:],
                    start=False, stop=(lt == 1),
                )
            ysbf = yppool.tile([128, L], F32, tag="ysbs", name="ysbf_t")
            nc.vector.tensor_copy(ysbf[:], psf[:])
            for nb2 in range(2):
                eng = nc.scalar if (nb2 == 1 and g == NG - 1) else nc.sync
                eng.dma_start(ypv[g, S - 1][nb2][:, mt, :], ysbf[ts(nb2, 64), :])

        def phase23(g, filler=None):
            # block-pair mt=0 needs ys blocks 0,1 (ready after p2 step 0);
            # mt=1 needs blocks 2,3 (after steps 1,2).  p2 step 3 rolls the
            # carry into the next group.  The filler (an interleaved phase-1
            # chain) fills the PE stalls between serial p2 steps.
            p2step(g, 0)
            drain(filler, 1)
            p3_j0(g, 0)
            drain(filler, 1)
            for k in range(7):
                p3_pair(g, k, 0)
                drain(filler, 1)
            p3_j15(g, 0)
            drain(filler, 1)
            p2step(g, 1)
            drain(filler, 1)
            p2step(g, 2)
            drain(filler, 1)
            p3_j0(g, 1)
            drain(filler, 1)
            for k in range(7):
                p3_pair(g, k, 1)
                drain(filler, 1)
            p3_j15(g, 1)
            p2step(g, 3)
            drain(filler)

        # ------------------------------------------------------------------
        # Emit: chunk 0 first (PE starts ASAP), power-chain steps spread
        # between chunks, recurrence phases pipelined one group behind.
        # ------------------------------------------------------------------
        encoder_chunk(0, prep_hook=one_time_prep)
        for b in range(1, GB):
            load_sxu(b)
        emit_chain(2, 7)
        encoder_chunk(1)
        emit_chain(7, 12)
        encoder_chunk(2)
        emit_chain(12, S + 1)
        encoder_chunk(3)
        for g in range(1, NG):
            for big in range(GB):
                load_sxu(g * GB + big)
            for big in range(GB - 1):
                encoder_chunk(g * GB + big)
            f = phase1_gen(g - 1)
            encoder_chunk(g * GB + GB - 1, filler=f)
            drain(f)
            if g < NG - 1:
                phase23(g - 1)
            else:
                phase23(g - 1, filler=phase1_gen(NG - 1))
        phase23(NG - 1)

    nc.compile()
    return nc


_NC = None


def _get_nc():
    global _NC
    if _NC is None:
        _NC = _build()
    return _NC


def kernel(**inputs):
    nc = _get_nc()
    wnames = [
        "Wx1", "bx1", "Wx2", "bx2", "Wx3", "Wu1", "bu1", "Wu2", "WB", "WK", "bK",
    ]
    weights = {k: np.ascontiguousarray(np.asarray(inputs[k], dtype=np.float32)) for k in wnames}
    x = np.asarray(inputs["x"], dtype=np.float32)
    u = np.asarray(inputs["u"], dtype=np.float32)
    in_maps = []
    for c in range(NCORES):
        m = dict(weights)
        m["x"] = np.ascontiguousarray(x[c * BL : (c + 1) * BL])
        m["u"] = np.ascontiguousarray(u[c * BL : (c + 1) * BL])
        in_maps.append(m)
    res = run_bass_kernel_spmd(nc, in_maps, core_ids=list(range(NCORES)))
    y = np.concatenate([r["y"] for r in res.results], axis=0)
    y_pred = np.concatenate([r["y_pred"] for r in res.results], axis=0)
    return (y, y_pred)



# revision 50
# speedup vs baseline: 1.1984x; 1.0093x over previous
"""Trainium2 Bass kernel for nn_DEINA: encoder + Koopman linear recurrence.

Self-contained: shards the batch (512 trajectories) over 8 NeuronCores
(64 trajectories each), runs a fused encoder + blocked-scan recurrence
per core, and gathers the full outputs.

Math (per trajectory, T=256 steps, D=64, H=256, G=192, L=256):
    g  = relu(x Wx1 + bx1); g = relu(g Wx2 + bx2); g = g Wx3
    y  = [x, g]                                  (output 1)
    v  = relu(u Wu1 + bu1) Wu2;  uu = [u, v];  Bu = uu WB
    y_pred[0] = y[0];  y_pred[t+1] = y_pred[t] K + bK + Bu[t]   (output 2)

Key optimizations over the straightforward version:
  - Bu is computed without materializing v:  Bu = u WB[:64] + h1u (Wu2 WB[64:])
    with the fused weight WuB = Wu2 @ WB[64:] built once on device.
  - K powers P_j (j=1..16) are chained in f32r (full PE rate) and stored
    as [P_{2k+1} | P_{2k+2}] pairs so phase-3 matmuls run at N=512.
  - The y x-part is written DRAM->DRAM (no SBUF bounce).
  - The time recurrence is a blocked scan with S=16:
      phase 1: z[b,j] = z[b,j-1] K + c[b,j]  batched over blocks
      phase 2: 15 tiny serial steps through K^16
      phase 3: y_pred[b*16+j] = ys[b] K^j + z[b,j-1], two j's per matmul
"""

import numpy as np

import concourse.bacc as bacc
import concourse.bass as bass
import concourse.tile as tile
from concourse import mybir
from concourse.bass import ts
from concourse.bass_utils import run_bass_kernel_spmd
from concourse.masks import make_identity

F32 = mybir.dt.float32
F32R = mybir.dt.float32r
BF16 = mybir.dt.bfloat16
RELU = mybir.ActivationFunctionType.Relu

NCORES = 8
BL = 64  # trajectories per core
T = 256
D = 64
H = 256
G = 192
L = 256
S = 16  # scan block size (= time steps per chunk)
NB = 16  # number of blocks
NG = 4  # block groups
GB = 4  # blocks per group


def _build():
    nc = bacc.Bacc("TRN2", target_bir_lowering=False)

    x_h = nc.dram_tensor("x", [BL, T, D], F32, kind="ExternalInput")
    u_h = nc.dram_tensor("u", [BL, T, D], F32, kind="ExternalInput")
    wx1_h = nc.dram_tensor("Wx1", [D, H], F32, kind="ExternalInput")
    bx1_h = nc.dram_tensor("bx1", [H], F32, kind="ExternalInput")
    wx2_h = nc.dram_tensor("Wx2", [H, H], F32, kind="ExternalInput")
    bx2_h = nc.dram_tensor("bx2", [H], F32, kind="ExternalInput")
    wx3_h = nc.dram_tensor("Wx3", [H, G], F32, kind="ExternalInput")
    wu1_h = nc.dram_tensor("Wu1", [D, H], F32, kind="ExternalInput")
    bu1_h = nc.dram_tensor("bu1", [H], F32, kind="ExternalInput")
    wu2_h = nc.dram_tensor("Wu2", [H, G], F32, kind="ExternalInput")
    wb_h = nc.dram_tensor("WB", [L, L], F32, kind="ExternalInput")
    wk_h = nc.dram_tensor("WK", [L, L], F32, kind="ExternalInput")
    bk_h = nc.dram_tensor("bK", [L], F32, kind="ExternalInput")
    y_h = nc.dram_tensor("y", [BL, T, L], F32, kind="ExternalOutput")
    yp_h = nc.dram_tensor("y_pred", [BL, T, L], F32, kind="ExternalOutput")

    with tile.TileContext(nc) as tc, tile.ExitStack() as ctx:
        wpool = ctx.enter_context(tc.tile_pool(name="w", bufs=1))
        encpool = ctx.enter_context(tc.tile_pool(name="enc", bufs=3))
        inppool = ctx.enter_context(tc.tile_pool(name="inp", bufs=3))
        actpool = ctx.enter_context(tc.tile_pool(name="act", bufs=2))
        czpool = ctx.enter_context(tc.tile_pool(name="cz", bufs=2))
        yspool = ctx.enter_context(tc.tile_pool(name="ys", bufs=2))
        yppool = ctx.enter_context(tc.tile_pool(name="ypd", bufs=5))
        stgpool = ctx.enter_context(tc.tile_pool(name="stg", bufs=1))
        gspool = ctx.enter_context(tc.tile_pool(name="gs", bufs=3))
        encps = ctx.enter_context(tc.tile_pool(name="encps", bufs=2, space="PSUM"))
        tpps = ctx.enter_context(tc.tile_pool(name="tpps", bufs=1, space="PSUM"))
        sps = ctx.enter_context(tc.tile_pool(name="sps", bufs=3, space="PSUM"))

        # ------------------------------------------------------------------
        # Input chunk 0 first so the PE can start transposing ASAP
        # ------------------------------------------------------------------
        sxu_tiles = {}

        def load_sxu(b):
            t = encpool.tile([BL, S, 2 * D], BF16, tag="sxu", name="sxu")
            nc.gpsimd.dma_start(t[:, :, 0:D], x_h[:, ts(b, S), :])
            nc.gpsimd.dma_start(t[:, :, D : 2 * D], u_h[:, ts(b, S), :])
            sxu_tiles[b] = t
            return t

        load_sxu(0)

        # Identities early (chunk-0 transposes need identb64, and the DVE
        # FIFO must not park these casts behind weight casts), plus a run
        # of throwaway PE transposes: the PE clock sits at 1.2 GHz until
        # ~3.4us of sustained activity (HAM); warming it while the input
        # DMAs are in flight makes the real matmuls start at 2.4 GHz.
        ident = wpool.tile([128, 128], F32, tag="ident")
        make_identity(nc, ident[:])
        identrt = wpool.tile([128, 128], F32R, tag="identrt")
        nc.vector.tensor_copy(identrt[:], ident[:])
        identb64 = wpool.tile([64, 64], BF16, tag="identb64")
        nc.vector.tensor_copy(identb64[:], ident[0:64, 0:64])
        identb128 = wpool.tile([128, 128], BF16, tag="identb128")
        nc.vector.tensor_copy(identb128[:], ident[:])

        # ------------------------------------------------------------------
        # Weights / constants (issued from several queues in parallel)
        # ------------------------------------------------------------------
        def load_f32(ap, shape, name, pool=wpool, eng=None):
            t = pool.tile(shape, F32, tag=name, name=name)
            (eng or nc.sync).dma_start(t[:], ap)
            return t

        def to_bf16(src, name):
            t = wpool.tile(list(src.shape), BF16, tag=name, name=name)
            nc.vector.tensor_copy(t[:], src[:])
            return t

        # L1 weights: wx1 lives on partitions 0:64, wu1 on 64:128
        wx1f = load_f32(wx1_h[:, :], [D, H], "wx1f", pool=stgpool)
        wx1b = to_bf16(wx1f, "wx1b")
        wu1f = stgpool.tile([128, H], F32, tag="wu1f")
        nc.scalar.dma_start(wu1f[64:128, :], wu1_h[:, :])
        wu1b = wpool.tile([128, H], BF16, tag="wu1b")
        nc.vector.tensor_copy(wu1b[64:128, :], wu1f[64:128, :])

        wx2b, wx3b = [], []
        for lt in range(2):
            wx2b.append(to_bf16(load_f32(wx2_h.ap()[ts(lt, 128), :], [128, H], f"wx2f{lt}", pool=stgpool), f"wx2b{lt}"))
            wx3b.append(to_bf16(load_f32(wx3_h.ap()[ts(lt, 128), :], [128, G], f"wx3f{lt}", pool=stgpool, eng=nc.scalar), f"wx3b{lt}"))

        # WB split: wb1 = WB[0:64] (u-part, on partitions 64:128);
        # wb2 = WB[64:256] (v-part, fused into WuB below)
        wb1f = stgpool.tile([128, L], F32, tag="wb1f")
        nc.scalar.dma_start(wb1f[64:128, :], wb_h.ap()[0:64, :])
        wb1b = wpool.tile([128, L], BF16, tag="wb1b")
        nc.vector.tensor_copy(wb1b[64:128, :], wb1f[64:128, :])
        wb2f = [
            load_f32(wb_h.ap()[64:192, :], [128, L], "wb2f0", pool=stgpool, eng=nc.scalar),
            load_f32(wb_h.ap()[192:256, :], [64, L], "wb2f1", pool=stgpool, eng=nc.scalar),
        ]
        wu2f = [load_f32(wu2_h.ap()[ts(lt, 128), :], [128, G], f"wu2f{lt}", pool=stgpool, eng=nc.scalar) for lt in range(2)]

        # biases as per-partition scalars: col j holds b[j*128 + p]
        def load_bias(h, name):
            t = wpool.tile([128, 2], F32, tag=name, name=name)
            nc.sync.dma_start(t[:], h.rearrange("(t p) -> p t", p=128))
            return t

        bx1v = load_bias(bx1_h, "bx1v")
        bx2v = load_bias(bx2_h, "bx2v")
        bu1v = load_bias(bu1_h, "bu1v")
        bkv = load_bias(bk_h, "bkv")

        # K tiles (f32)
        kf = [load_f32(wk_h.ap()[ts(lt, 128), :], [128, L], f"kf{lt}") for lt in range(2)]

        # ------------------------------------------------------------------
        # Views for strided HBM I/O
        # ------------------------------------------------------------------
        # g-part of y: one DMA per chunk; rows (j2, traj), free (mt, l)
        yv_g = y_h.rearrange("traj (b mt j2) l -> b j2 traj mt l", b=NB, mt=8, j2=2)
        # y_pred singles (j=0): rows (nb2, traj), free (mt, l)
        ypv = yp_h.rearrange(
            "traj (g mt nb2 j) l -> g j nb2 traj mt l", g=NG, mt=2, nb2=2, j=S
        )
        # y_pred pairs: per (g, mt): dims (nb2, traj, j, l)
        ypv2 = yp_h.rearrange(
            "traj (g mt nb2 j) l -> g mt nb2 traj j l", g=NG, mt=2, nb2=2, j=S
        )

        cz = {}  # (group, lt) -> [128, S, GB, BL] f32 tile
        ys = {}  # (group, lt) -> [128, GB, BL] f32 tile

        def get_cz(g, lt):
            if (g, lt) not in cz:
                cz[(g, lt)] = czpool.tile([128, S, GB, BL], BF16, tag=f"cz{lt}", name=f"cz{g}_{lt}")
            return cz[(g, lt)]

        def get_ys(g, lt):
            if (g, lt) not in ys:
                ys[(g, lt)] = yspool.tile([128, GB, BL], F32R, tag=f"ys{lt}", name=f"ys{g}_{lt}")
            return ys[(g, lt)]

        # ------------------------------------------------------------------
        # Encoder chunk: one block b (16 time steps x 64 trajectories),
        # processed as one 1024-token wave (N=1024 matmuls)
        # ------------------------------------------------------------------
        def drain(gen, n=None):
            if gen is None:
                return
            try:
                if n is None:
                    while True:
                        next(gen)
                else:
                    for _ in range(n):
                        next(gen)
            except StopIteration:
                pass

        def encoder_chunk(b, filler=None, prep_hook=None):
            g = b // GB
            big = b % GB
            sxu = sxu_tiles[b]
            # y x-part: straight DRAM->DRAM passthrough
            nc.gpsimd.dma_start(y_h[:, ts(b, S), 0:D], x_h[:, ts(b, S), :])

            # PE-transpose the 16 [64,128] t-slabs into one psum bank:
            # partitions (x-d | u-d), cols (t, traj)
            xps = tpps.tile([128, S * BL], BF16, tag="tpps", name="tpps_t")
            for t in range(S):
                nc.tensor.matmul(
                    xps[:, ts(t, BL)], sxu[:, t, :], identb64[:],
                    is_transpose=True, start=(t == 0), stop=(t == S - 1),
                )
            xu = encpool.tile([128, S * BL], BF16, tag="xu", name="xu")
            nc.any.tensor_copy(xu[:], xps[:])
            drain(filler, 2)

            rx = xu[0:D, :]
            ru = xu[D:128, :]
            NTOK = S * BL  # 1024

            # L1: h1x = relu(Wx1^T x^T + bx1), h1u likewise (K=64)
            h1xs, h1us = [], []
            for mt in range(2):
                psx = encps.tile([128, NTOK], F32, tag="encps", name="encps_t")
                psu = encps.tile([128, NTOK], F32, tag="encps", name="encps_t")
                for hf in range(2):
                    # x on array rows 0:64, u on rows 64:128 -> concurrent
                    nc.tensor.matmul(
                        psx[:, ts(hf, 512)], wx1b[:, ts(mt, 128)], rx[:, ts(hf, 512)],
                        start=True, stop=True,
                    )
                    nc.tensor.matmul(
                        psu[:, ts(hf, 512)], wu1b[64:128, ts(mt, 128)], ru[:, ts(hf, 512)],
                        start=True, stop=True, tile_position=(64, 0),
                    )
                sbx = actpool.tile([128, NTOK], BF16, tag=f"h1x{mt}", name=f"h1x{mt}_t")
                nc.scalar.activation(sbx[:], psx[:], RELU, bias=bx1v[:, mt : mt + 1])
                h1xs.append(sbx)
                sbu = actpool.tile([128, NTOK], BF16, tag=f"h1u{mt}", name=f"h1u{mt}_t")
                nc.vector.tensor_scalar(
                    sbu[:], psu[:], bu1v[:, mt : mt + 1], 0.0,
                    op0=mybir.AluOpType.add, op1=mybir.AluOpType.max,
                )
                h1us.append(sbu)
                drain(filler, 2)

            # L2: h2x = relu(Wx2^T h1x + bx2)
            h2xs = []
            for mt in range(2):
                ps = encps.tile([128, NTOK], F32, tag="encps", name="encps_t")
                for hf in range(2):
                    for lt in range(2):
                        nc.tensor.matmul(
                            ps[:, ts(hf, 512)], wx2b[lt][:, ts(mt, 128)],
                            h1xs[lt][:, ts(hf, 512)],
                            start=(lt == 0), stop=(lt == 1),
                        )
                sb = actpool.tile([128, NTOK], BF16, tag=f"h2x{mt}", name=f"h2x{mt}_t")
                nc.scalar.activation(sb[:], ps[:], RELU, bias=bx2v[:, mt : mt + 1])
                h2xs.append(sb)
                drain(filler, 2)

            if prep_hook is not None:
                prep_hook()

            # Bu + bK -> c buffer, via fused WuB (no v materialization):
            # Bu = u @ WB[0:64] + h1u @ WuB
            for mt in range(2):
                ps = encps.tile([128, NTOK], F32, tag="encps", name="encps_t")
                for hf in range(2):
                    nc.tensor.matmul(
                        ps[:, ts(hf, 512)], wb1b[64:128, ts(mt, 128)],
                        ru[:, ts(hf, 512)],
                        start=True, stop=False, tile_position=(64, 0),
                    )
                    for lt in range(2):
                        nc.tensor.matmul(
                            ps[:, ts(hf, 512)], wuBb[lt][:, ts(mt, 128)],
                            h1us[lt][:, ts(hf, 512)],
                            start=False, stop=(lt == 1),
                        )
                czt = get_cz(g, mt)
                nc.vector.tensor_scalar_add(
                    czt[:, :, big, :],
                    ps[:].rearrange("p (a c) -> p a c", a=S),
                    bkv[:, mt : mt + 1],
                )
                drain(filler, 2)

            # g (natural layout) = h2x @ Wx3: 8 M-tiles in 2 psum tiles
            gps = [encps.tile([128, NTOK], F32, tag="encps", name="gps_t") for _ in range(2)]
            drain(filler, 1)
            for mt8 in range(8):
                out = gps[mt8 // 4][:, (mt8 % 4) * 256 : (mt8 % 4) * 256 + G]
                for lt in range(2):
                    nc.tensor.matmul(
                        out, h2xs[lt][:, ts(mt8, 128)], wx3b[lt][:],
                        start=(lt == 0), stop=(lt == 1),
                    )
            gs = gspool.tile([128, 8, G], F32, tag="gs", name="gs_t")
            nc.scalar.copy(
                gs[:, ts(0, 4), :],
                gps[0][:].rearrange("p (m x) -> p m x", m=4)[:, :, 0:G],
            )
            nc.vector.tensor_copy(
                gs[:, ts(1, 4), :],
                gps[1][:].rearrange("p (m x) -> p m x", m=4)[:, :, 0:G],
            )
            for j2 in range(2):
                nc.gpsimd.dma_start(yv_g[b, j2][:, :, D:L], gs[ts(j2, 64), :, :])

            # y0 (t = 0): x-part copied from xu, g-part via matmuls
            if b == 0:
                nc.vector.tensor_copy(get_ys(0, 0)[0:D, 0, :], xu[0:D, 0:BL])
                y0a = sps.tile([128, BL], F32, tag="sps", name="y0a_t")
                for lt in range(2):
                    nc.tensor.matmul(
                        y0a[64:128, :], wx3b[lt][:, 0:64], h2xs[lt][:, 0:BL],
                        start=(lt == 0), stop=(lt == 1), tile_position=(0, 64),
                    )
                nc.vector.tensor_copy(get_ys(0, 0)[64:128, 0, :], y0a[64:128, :])
                y0b = sps.tile([128, BL], F32, tag="sps", name="y0b_t")
                for lt in range(2):
                    nc.tensor.matmul(
                        y0b[:], wx3b[lt][:, 64:192], h2xs[lt][:, 0:BL],
                        start=(lt == 0), stop=(lt == 1),
                    )
                nc.vector.tensor_copy(get_ys(0, 1)[:, 0, :], y0b[:])

        # ------------------------------------------------------------------
        # One-time weight prep on PE: K^T, fused WuB, K powers (f32r chain).
        # Emitted before chunk 0 (its Bu needs WuB); the serial power chain
        # is spread between chunk emissions so the in-order PE queue never
        # stalls on its psum->sbuf copy latency.
        # ------------------------------------------------------------------
        # K^T tiles (for the power chain): kT[b][p, a] = K[a, b*128+p]
        kT = [wpool.tile([128, L], F32R, tag=f"kT{lt}", name=f"kT{lt}") for lt in range(2)]
        wu2T0 = stgpool.tile([128, H], F32, tag="wu2T0")
        wu2T1 = stgpool.tile([64, H], F32, tag="wu2T1")
        wuBb = []
        kb1 = [wpool.tile([128, L], BF16, tag=f"kb1_{rt}", name=f"kb1_{rt}") for rt in range(2)]
        prp = {}
        for rt in range(2):
            for k in range(8):
                prp[(k, rt)] = wpool.tile([128, 2 * L], F32R, tag=f"prp{k}_{rt}", name=f"prp{k}_{rt}")

        def one_time_prep():
            for a in range(2):
                for bb in range(2):
                    pst = sps.tile([128, 128], F32, tag="sps", name="pstT_t")
                    nc.tensor.transpose(pst[:], kf[a][:, ts(bb, 128)], ident[:])
                    nc.vector.tensor_copy(kT[bb][:, ts(a, 128)], pst[:])

            # Wu2^T (for the WuB build)
            for ht in range(2):
                p0 = sps.tile([128, 128], F32, tag="sps", name="wu2t_t")
                nc.tensor.transpose(p0[:], wu2f[ht][:, 0:128], ident[:])
                nc.scalar.copy(wu2T0[:, ts(ht, 128)], p0[:])
                p1 = sps.tile([128, 128], F32, tag="sps", name="wu2t_t")
                nc.tensor.transpose(p1[0:64, :], wu2f[ht][:, 128:192], ident[:])
                nc.scalar.copy(wu2T1[:, ts(ht, 128)], p1[0:64, :])

            # WuB = Wu2 @ WB[64:256]  (bf16, 2 row tiles)
            for mt in range(2):
                ps = sps.tile([128, L], F32, tag="sps", name="wuB_t")
                nc.tensor.matmul(
                    ps[:], wu2T0[:, ts(mt, 128)], wb2f[0][:],
                    start=True, stop=False,
                )
                nc.tensor.matmul(
                    ps[:], wu2T1[:, ts(mt, 128)], wb2f[1][:],
                    start=False, stop=True,
                )
                wb_t = wpool.tile([128, L], BF16, tag=f"wuBb{mt}", name=f"wuBb{mt}")
                nc.scalar.copy(wb_t[:], ps[:])
                wuBb.append(wb_t)

            # K powers pair-store init: P_1 = K; kb1 = bf16 K for phase 1
            for rt in range(2):
                nc.vector.tensor_copy(prp[(0, rt)][:, 0:L], kf[rt][:])
                nc.scalar.copy(kb1[rt][:], kf[rt][:])

        def pslice(j, rt):  # P_j for row-tile rt
            k, c = (j - 1) // 2, (j - 1) % 2
            return prp[(k, rt)][:, c * L : (c + 1) * L]

        def emit_chain(j0, j1):
            for j in range(j0, j1):
                for rt in range(2):
                    pst = sps.tile([128, L], F32, tag="sps", name="pstP_t")
                    for bt in range(2):
                        nc.tensor.matmul(
                            pst[:],
                            kT[bt][:, ts(rt, 128)],
                            pslice(j - 1, bt),
                            start=(bt == 0),
                            stop=(bt == 1),
                        )
                    nc.vector.tensor_copy(pslice(j, rt), pst[:])

        # ------------------------------------------------------------------
        # Phase 1: batched local scans (per group)
        # ------------------------------------------------------------------
        def phase1_gen(g):
            czt = [get_cz(g, lt) for lt in range(2)]
            for j in range(1, S):
                zprev = [czt[lt][:, j - 1, :, :].rearrange("p a c -> p (a c)") for lt in range(2)]
                ps = sps.tile([128, 512], F32, tag="sps", name="p1ps_t")
                for l2t in range(2):
                    for l1t in range(2):
                        nc.tensor.matmul(
                            ps[:, ts(l2t, GB * BL)],
                            kb1[l1t][:, ts(l2t, 128)],
                            zprev[l1t],
                            start=(l1t == 0 and l2t == 0),
                            stop=(l1t == 1 and l2t == 1),
                        )
                for l2t in range(2):
                    nc.vector.tensor_add(
                        czt[l2t][:, j, :, :],
                        ps[:, ts(l2t, GB * BL)].rearrange("p (b c) -> p b c", b=GB),
                        czt[l2t][:, j, :, :],
                    )
                yield

        # ------------------------------------------------------------------
        # Phase 2: block-level scan (serial, 4 steps per group)
        # ------------------------------------------------------------------
        def p2step(g, nb):
            b = g * GB + nb
            if b >= NB - 1:
                return
            ng, nnb = (g, nb + 1) if nb + 1 < GB else (g + 1, 0)
            ps = sps.tile([128, 2 * BL], F32, tag="sps", name="p2ps_t")
            for lt in range(2):
                for l1t in range(2):
                    nc.tensor.matmul(
                        ps[:, ts(lt, BL)],
                        pslice(S, l1t)[:, ts(lt, 128)],
                        get_ys(g, l1t)[:, nb, :],
                        start=(l1t == 0 and lt == 0),
                        stop=(l1t == 1 and lt == 1),
                    )
            for lt in range(2):
                nc.vector.tensor_add(
                    get_ys(ng, lt)[:, nnb, :], ps[:, ts(lt, BL)],
                    get_cz(g, lt)[:, S - 1, nb, :],
                )

        # ------------------------------------------------------------------
        # Phase 3: fix-up, natural-layout output.
        # j = 0: transpose-only.  j in {1..14}: pairs (2k+1, 2k+2) with
        # N=512 matmuls against the prp pair tiles.  j = 15: single.
        # ------------------------------------------------------------------
        def p3_j0(g, mt):
            ps0 = sps.tile([128, L], F32, tag="sps", name="p3ps0_t")
            for lt in range(2):
                nc.tensor.matmul(
                    ps0[:, lt * 128 : lt * 128 + 128].bitcast(F32R),
                    get_ys(g, lt)[:, ts(mt, 2), :].rearrange("p a c -> p (a c)"),
                    identrt[:],
                    is_transpose=True, start=(lt == 0), stop=(lt == 1),
                )
            ysb0 = yppool.tile([128, L], F32, tag="ysbs", name="ysb0_t")
            nc.vector.tensor_copy(ysb0[:], ps0[:])
            for nb2 in range(2):
                eng = nc.scalar if (nb2 == 1 and g == NG - 1) else nc.sync
                eng.dma_start(ypv[g, 0][nb2][:, mt, :], ysb0[ts(nb2, 64), :])

        def p3_pair(g, k, mt):
            ps = sps.tile([128, 2 * L], F32, tag="sps", name="p3ps_t")
            for l1t in range(2):
                nc.tensor.matmul(
                    ps[:],
                    get_ys(g, l1t)[:, ts(mt, 2), :].rearrange("p a c -> p (a c)"),
                    prp[(k, l1t)][:],
                    start=(l1t == 0), stop=False,
                )
            for jc in range(2):
                for lt in range(2):
                    nc.tensor.matmul(
                        ps[:, jc * L + lt * 128 : jc * L + lt * 128 + 128],
                        get_cz(g, lt)[:, 2 * k + jc, ts(mt, 2), :].rearrange("p a c -> p (a c)"),
                        identb128[:],
                        start=False, stop=(jc == 1 and lt == 1),
                    )
            ysb = yppool.tile([128, 2, L], F32, tag="ysb", name="ysb_t")
            # last group: ACT's queue also issues the scalar-half output
            # DMAs; keep its copies on DVE there so they don't serialize
            if g == NG - 1:
                nc.vector.tensor_copy(ysb[:], ps[:].rearrange("p (m x) -> p m x", m=2))
            else:
                nc.scalar.copy(ysb[:], ps[:].rearrange("p (m x) -> p m x", m=2))
            for nb2 in range(2):
                eng = nc.scalar if (nb2 == 1 and g == NG - 1) else nc.sync
                eng.dma_start(
                    ypv2[g, mt][nb2][:, 2 * k + 1 : 2 * k + 3, :],
                    ysb[ts(nb2, 64), :, :],
                )

        def p3_j15(g, mt):
            psf = sps.tile([128, L], F32, tag="sps", name="p3psf_t")
            for l1t in range(2):
                nc.tensor.matmul(
                    psf[:],
                    get_ys(g, l1t)[:, ts(mt, 2), :].rearrange("p a c -> p (a c)"),
                    pslice(S - 1, l1t),
                    start=(l1t == 0), stop=False,
                )
            for lt in range(2):
                nc.tensor.matmul(
                    psf[:, lt * 128 : lt * 128 + 128],
                    get_cz(g, lt)[:, S - 2, ts(mt, 2), :].rearrange("p a c -> p (a c)"),
                    identb128[:],
                    start=False, stop=(lt == 1),
                )
            ysbf = yppool.tile([128, L], F32, tag="ysbs", name="ysbf_t")
            nc.vector.tensor_copy(ysbf[:], psf[:])
            for nb2 in range(2):
                eng = nc.scalar if (nb2 == 1 and g == NG - 1) else nc.sync
                eng.dma_start(ypv[g, S - 1][nb2][:, mt, :], ysbf[ts(nb2, 64), :])

        def phase23(g, filler=None):
            # block-pair mt=0 needs ys blocks 0,1 (ready after p2 step 0);
            # mt=1 needs blocks 2,3 (after steps 1,2).  p2 step 3 rolls the
            # carry into the next group.  The filler (an interleaved phase-1
            # chain) fills the PE stalls between serial p2 steps.
            p2step(g, 0)
            drain(filler, 1)
            p3_j0(g, 0)
            drain(filler, 1)
            for k in range(7):
                p3_pair(g, k, 0)
                drain(filler, 1)
            p3_j15(g, 0)
            drain(filler, 1)
            p2step(g, 1)
            drain(filler, 1)
            p2step(g, 2)
            drain(filler, 1)
            p3_j0(g, 1)
            drain(filler, 1)
            for k in range(7):
                p3_pair(g, k, 1)
                drain(filler, 1)
            p3_j15(g, 1)
            p2step(g, 3)
            drain(filler)

        # ------------------------------------------------------------------
        # Emit: chunk 0 first (PE starts ASAP), power-chain steps spread
        # between chunks, recurrence phases pipelined one group behind.
        # ------------------------------------------------------------------
        encoder_chunk(0, prep_hook=one_time_prep)
        for b in range(1, GB):
            load_sxu(b)
        emit_chain(2, 7)
        encoder_chunk(1)
        emit_chain(7, 12)
        encoder_chunk(2)
        emit_chain(12, S + 1)
        encoder_chunk(3)
        for g in range(1, NG):
            for big in range(GB):
                load_sxu(g * GB + big)
            for big in range(GB - 1):
                encoder_chunk(g * GB + big)
            f = phase1_gen(g - 1)
            encoder_chunk(g * GB + GB - 1, filler=f)
            drain(f)
            if g < NG - 1:
                phase23(g - 1)
            else:
                phase23(g - 1, filler=phase1_gen(NG - 1))
        phase23(NG - 1)

    nc.compile()
    return nc


_NC = None


def _get_nc():
    global _NC
    if _NC is None:
        _NC = _build()
    return _NC


def kernel(**inputs):
    nc = _get_nc()
    wnames = [
        "Wx1", "bx1", "Wx2", "bx2", "Wx3", "Wu1", "bu1", "Wu2", "WB", "WK", "bK",
    ]
    weights = {k: np.ascontiguousarray(np.asarray(inputs[k], dtype=np.float32)) for k in wnames}
    x = np.asarray(inputs["x"], dtype=np.float32)
    u = np.asarray(inputs["u"], dtype=np.float32)
    in_maps = []
    for c in range(NCORES):
        m = dict(weights)
        m["x"] = np.ascontiguousarray(x[c * BL : (c + 1) * BL])
        m["u"] = np.ascontiguousarray(u[c * BL : (c + 1) * BL])
        in_maps.append(m)
    res = run_bass_kernel_spmd(nc, in_maps, core_ids=list(range(NCORES)))
    y = np.concatenate([r["y"] for r in res.results], axis=0)
    y_pred = np.concatenate([r["y_pred"] for r in res.results], axis=0)
    return (y, y_pred)

